# revision 7
# baseline (speedup 1.0000x reference)
"""Multi-head attention on 8 Trainium2 NeuronCores (Bass/Tile).

Sharding: batch B=4 x head-groups 2 -> 8 cores. Each core computes full
attention for 1 batch element and 8 of 16 heads, producing a partial
output projection (Wo row-sharded); host sums the two partials per batch.

Device dataflow (per core), everything in "transposed" orientation so the
contraction dim always sits on SBUF partitions:
  qT/kT/vT [DM=1024, S=2048] fp32 from host (host pre-transposes).
  QT = (Wq^T qT) [512, S]  (ACT adds bq per-partition)   -> fp32r
  KT likewise; V natural [S, 512] via lhsT=vT s-tiles (bv added with a
  K=1 ones-row matmul), stored per head with an appended ones column:
  Vp [k-tile, head, 65] bf16.
  scores^T[k,q] = (K_h^T tile).T @ Q_h^T  (K=64 contraction; even/odd
  heads live on partitions 0-63/64-127 -> concurrent PE row-tiles).
  expS = Exp(scores * 0.125) ACT PSUM->SBUF bf16.
  PV: out'[65, q] += Vp_tile.T @ expS_tile accumulated over 16 k-tiles;
  row 64 is the softmax denominator (ones column trick).
  A^T = out'[0:64] * reciprocal(out'[64]) broadcast -> fp32r [512, S].
  O^T[m, s] = Wo_chunk.T @ A^T chunk accumulated over 4 chunks -> fp32.
Host: out[b] = (O^T_hg0 + O^T_hg1).T + bo.
"""

import sys

sys.path.insert(0, "/opt/trn_rl_repo")

import numpy as np

import concourse.bacc as bacc
import concourse.mybir as mybir
from concourse import tile
from concourse.bass_utils import run_bass_kernel_spmd

F32 = mybir.dt.float32
F32R = mybir.dt.float32r
BF16 = mybir.dt.bfloat16
AF = mybir.ActivationFunctionType

H, DK, DV, DM = 16, 64, 64, 1024
B, S = 4, 2048
HL = H // 2          # heads per core
NB = HL * DK         # 512: per-core projection width
NDM = DM // 128      # 8 contraction chunks
NT = NB // 128       # 4 row-tiles of QT/KT/AT
NSB = S // 512       # 4 s-blocks
NKT = S // 128       # 16 k-tiles
SCALE = 1.0 / 8.0    # 1/sqrt(DK)

_CACHED_NC = None


def _build():
    nc = bacc.Bacc("TRN2", debug=False)

    qT = nc.dram_tensor("qT", [DM, S], F32, kind="ExternalInput")
    kT = nc.dram_tensor("kT", [DM, S], F32, kind="ExternalInput")
    vT = nc.dram_tensor("vT", [DM, S], F32, kind="ExternalInput")
    wq = nc.dram_tensor("wq", [DM, NB], F32, kind="ExternalInput")
    wk = nc.dram_tensor("wk", [DM, NB], F32, kind="ExternalInput")
    wv = nc.dram_tensor("wv", [DM, NB], F32, kind="ExternalInput")
    wo = nc.dram_tensor("wo", [NB, DM], F32, kind="ExternalInput")
    bq = nc.dram_tensor("bq", [NB], F32, kind="ExternalInput")
    bk = nc.dram_tensor("bk", [NB], F32, kind="ExternalInput")
    bv = nc.dram_tensor("bv", [NB], F32, kind="ExternalInput")
    ones = nc.dram_tensor("ones", [1, 128], F32, kind="ExternalInput")
    outT = nc.dram_tensor("outT", [DM, S], F32, kind="ExternalOutput")

    with tile.TileContext(nc) as tc:
        with tc.tile_pool(name="persist", bufs=1) as persist:
            QT = persist.tile([128, NT, S], F32R)
            KT = persist.tile([128, NT, S], F32R)
            Vp = persist.tile([128, NKT, HL, DV + 1], BF16)
            wo_sb = persist.tile([128, NT, DM], F32R)
            bq_sb = persist.tile([128, NT], F32)
            bk_sb = persist.tile([128, NT], F32)
            bv_sb = persist.tile([1, NB], F32R)
            ones_sb = persist.tile([1, 128], F32R)

            nc.sync.dma_start(
                wo_sb[:], wo.rearrange("(c p) m -> p c m", p=128).bitcast(F32R)
            )
            nc.sync.dma_start(bq_sb[:], bq.rearrange("(t p) -> p t", p=128))
            nc.sync.dma_start(bk_sb[:], bk.rearrange("(t p) -> p t", p=128))
            nc.sync.dma_start(
                bv_sb[:], bv.rearrange("(o n) -> o n", o=1).bitcast(F32R)
            )
            nc.sync.dma_start(ones_sb[:], ones[:].bitcast(F32R))
            nc.vector.memset(Vp[:, :, :, DV : DV + 1], 1.0)

            # ---- Stage 1: projections ----
            with (
                tc.tile_pool(name="acts", bufs=3) as acts_pool,
                tc.tile_pool(name="wgt", bufs=2) as wgt_pool,
                tc.tile_pool(name="ps_proj", bufs=2, space="PSUM") as ps_proj,
            ):
                for src_act, src_w, bias_sb, dstT in (
                    (qT, wq, bq_sb, QT),
                    (kT, wk, bk_sb, KT),
                ):
                    wt = wgt_pool.tile([128, NDM, NB], F32R, tag="w")
                    nc.sync.dma_start(
                        wt[:],
                        src_w.rearrange("(c p) n -> p c n", p=128).bitcast(F32R),
                    )
                    for sq in range(NSB):
                        act = acts_pool.tile([128, NDM, 512], F32R, tag="a")
                        nc.sync.dma_start(
                            act[:],
                            src_act[:, sq * 512 : (sq + 1) * 512]
                            .rearrange("(c p) s -> p c s", p=128)
                            .bitcast(F32R),
                        )
                        for t in range(NT):
                            ps = ps_proj.tile([128, 512], F32)
                            for c in range(NDM):
                                nc.tensor.matmul(
                                    ps[:],
                                    wt[:, c, t * 128 : (t + 1) * 128],
                                    act[:, c, :],
                                    start=(c == 0),
                                    stop=(c == NDM - 1),
                                )
                            nc.scalar.activation(
                                dstT[:, t, sq * 512 : (sq + 1) * 512],
                                ps[:],
                                AF.Identity,
                                bias=bias_sb[:, t : t + 1],
                            )

                # V projection: natural orientation, bias via ones-row matmul
                wt = wgt_pool.tile([128, NDM, NB], F32R, tag="w")
                nc.sync.dma_start(
                    wt[:], wv.rearrange("(c p) n -> p c n", p=128).bitcast(F32R)
                )
                for sq in range(NSB):
                    act = acts_pool.tile([128, NDM, 512], F32R, tag="a")
                    nc.sync.dma_start(
                        act[:],
                        vT[:, sq * 512 : (sq + 1) * 512]
                        .rearrange("(c p) s -> p c s", p=128)
                        .bitcast(F32R),
                    )
                    for sti in range(4):
                        st = sq * 4 + sti
                        ps = ps_proj.tile([128, 512], F32)
                        for c in range(NDM):
                            nc.tensor.matmul(
                                ps[:],
                                act[:, c, sti * 128 : (sti + 1) * 128],
                                wt[:, c, :],
                                start=(c == 0),
                                stop=False,
                            )
                        nc.tensor.matmul(
                            ps[:],
                            ones_sb[0:1, :],
                            bv_sb[0:1, :],
                            start=False,
                            stop=True,
                        )
                        nc.vector.tensor_copy(
                            Vp[:, st, :, 0:DV],
                            ps[:].rearrange("p (h d) -> p h d", h=HL),
                        )

            # ---- Stage 2: attention ----
            with tc.tile_pool(name="att", bufs=1) as att_pool:
                AT = att_pool.tile([128, NT, S], F32R)
                with (
                    tc.tile_pool(name="expS", bufs=34) as exp_pool,
                    tc.tile_pool(name="rec", bufs=3) as rec_pool,
                    tc.tile_pool(name="ps_sc", bufs=4, space="PSUM") as ps_sc,
                    tc.tile_pool(name="ps_pv", bufs=4, space="PSUM") as ps_pv,
                ):
                    for hp in range(HL // 2):  # head pairs: even head on
                        t = hp                 # partitions 0-63, odd on 64-127
                        for qb in range(NSB):
                            qsl = slice(qb * 512, (qb + 1) * 512)
                            # scores + exp for both heads of the pair first;
                            # PV matmuls follow so the PE never waits on the
                            # exp of the tile it just produced.
                            ex_tiles = []
                            for kt in range(NKT):
                                for sub in range(2):
                                    psl = slice(sub * 64, sub * 64 + 64)
                                    scp = ps_sc.tile([128, 512], F32)
                                    nc.tensor.matmul(
                                        scp[:],
                                        KT[psl, t, kt * 128 : (kt + 1) * 128],
                                        QT[psl, t, qsl],
                                        start=True,
                                        stop=True,
                                    )
                                    ex = exp_pool.tile([128, 512], BF16, tag="e")
                                    nc.scalar.activation(
                                        ex[:], scp[:], AF.Exp, scale=SCALE
                                    )
                                    ex_tiles.append(ex)
                            pv_list = [
                                ps_pv.tile([128, 512], F32, tag="pv", name=f"pv{i}")
                                for i in range(2)
                            ]
                            for kt in range(NKT):
                                for sub in range(2):
                                    nc.tensor.matmul(
                                        pv_list[sub][0 : DV + 1, :],
                                        Vp[:, kt, hp * 2 + sub, :],
                                        ex_tiles[kt * 2 + sub][:],
                                        start=(kt == 0),
                                        stop=(kt == NKT - 1),
                                    )
                            for sub in range(2):
                                psl = slice(sub * 64, sub * 64 + 64)
                                pvp = pv_list[sub]
                                rec = rec_pool.tile([1, 512], F32, tag="r")
                                recb = rec_pool.tile([64, 512], F32, tag="rb")
                                nc.vector.reciprocal(rec[:], pvp[DV : DV + 1, :])
                                nc.gpsimd.partition_broadcast(recb[:], rec[:])
                                nc.vector.tensor_mul(
                                    AT[psl, t, qsl], pvp[0:DV, :], recb[:]
                                )

                # ---- Stage 3: output projection ----
                with (
                    tc.tile_pool(name="ostage", bufs=4) as ostage,
                    tc.tile_pool(name="ps_o", bufs=4, space="PSUM") as ps_o,
                ):
                    for m in range(NDM):
                        for sbk in range(NSB):
                            ps = ps_o.tile([128, 512], F32)
                            for cc in range(NT):
                                nc.tensor.matmul(
                                    ps[:],
                                    wo_sb[:, cc, m * 128 : (m + 1) * 128],
                                    AT[:, cc, sbk * 512 : (sbk + 1) * 512],
                                    start=(cc == 0),
                                    stop=(cc == NT - 1),
                                )
                            ot = ostage.tile([128, 512], F32, tag="o")
                            nc.vector.tensor_copy(ot[:], ps[:])
                            nc.sync.dma_start(
                                outT[
                                    m * 128 : (m + 1) * 128,
                                    sbk * 512 : (sbk + 1) * 512,
                                ],
                                ot[:],
                            )

    nc.compile()
    return nc


def get_nc():
    global _CACHED_NC
    if _CACHED_NC is None:
        _CACHED_NC = _build()
    return _CACHED_NC


def make_in_maps(queries, keys, values, Wq, bq, Wk, bk, Wv, bv, Wo, bo):
    queries = np.asarray(queries, np.float32)
    keys = np.asarray(keys, np.float32)
    values = np.asarray(values, np.float32)
    Wq = np.asarray(Wq, np.float32)
    Wk = np.asarray(Wk, np.float32)
    Wv = np.asarray(Wv, np.float32)
    Wo = np.asarray(Wo, np.float32)
    bq = np.asarray(bq, np.float32)
    bk = np.asarray(bk, np.float32)
    bv = np.asarray(bv, np.float32)
    ones = np.ones((1, 128), np.float32)
    in_maps = []
    for core in range(8):
        b, hg = divmod(core, 2)
        sl = slice(hg * NB, (hg + 1) * NB)
        in_maps.append(
            {
                "qT": np.ascontiguousarray(queries[b].T),
                "kT": np.ascontiguousarray(keys[b].T),
                "vT": np.ascontiguousarray(values[b].T),
                "wq": np.ascontiguousarray(Wq[:, sl]),
                "wk": np.ascontiguousarray(Wk[:, sl]),
                "wv": np.ascontiguousarray(Wv[:, sl]),
                "wo": np.ascontiguousarray(Wo[sl, :]),
                "bq": np.ascontiguousarray(bq[sl]),
                "bk": np.ascontiguousarray(bk[sl]),
                "bv": np.ascontiguousarray(bv[sl]),
                "ones": ones,
            }
        )
    return in_maps


def assemble(results, bo):
    bo = np.asarray(bo, np.float32)
    out = np.empty((B, S, DM), np.float32)
    for b in range(B):
        acc = results[2 * b]["outT"] + results[2 * b + 1]["outT"]
        out[b] = acc.T + bo
    return out


def run(trace=False, **inputs):
    if trace:
        # NTFF profiling shim: this image's antenv lacks axon_hooks.
        import types

        try:
            from antenv import axon_hooks  # noqa: F401
        except ImportError:
            from trn_agent_boot.trn_boot import _ntff_profile_via_ctypes

            mod = types.ModuleType("antenv.axon_hooks")
            _hook = _ntff_profile_via_ctypes("/opt/axon/libaxon_pjrt.so")
            mod.get_axon_ntff_profile_hook = lambda: _hook
            sys.modules["antenv.axon_hooks"] = mod
    nc = get_nc()
    bo = inputs["bo"]
    in_maps = make_in_maps(**inputs)
    res = run_bass_kernel_spmd(nc, in_maps, list(range(8)), trace=trace)
    return assemble(res.results, bo), res


def kernel(**inputs):
    out, _ = run(trace=False, **inputs)
    return out


# revision 17
# speedup vs baseline: 1.5213x; 1.5213x over previous
"""Multi-head attention on 8 Trainium2 NeuronCores (Bass/Tile).

Sharding: batch B=4 x head-groups 2 -> 8 cores. Each core computes full
attention for 1 batch element and 8 of 16 heads, producing a partial
output projection (Wo row-sharded); host sums the two partials per batch.

Device dataflow (per core), everything in "transposed" orientation so the
contraction dim always sits on SBUF partitions. All matmul operands are
bf16 (fp32 PSUM accumulation); fp32r measured 3x slower per column on HW.
  qT/kT/vT [DM=1024, S=2048] bf16 from host (host pre-transposes+casts).
  QT = (Wq^T qT) [512, S] bf16  (DVE adds bq while copying PSUM->SBUF)
  KT likewise; V natural [S, 512] via lhsT=vT s-tiles (bv added with a
  K=1 ones-row matmul), stored per head with an appended ones column:
  Vp [k-tile, head, 65] bf16.
  scores^T[k,q] = (K_h^T tile).T @ Q_h^T, N=1024 (K=64 contraction; even/
  odd heads on partitions 0-63/64-127 -> concurrent PE row-tiles).
  expS = Exp(scores * 0.125) ACT PSUM->SBUF bf16, [128,1024] chunks.
  PV: out'[65, q] += Vp_tile.T @ expS_tile accumulated over 16 k-tiles;
  row 64 is the softmax denominator (ones column trick).
  A^T = out'[0:64] * reciprocal(out'[64]) broadcast -> bf16 [512, S].
  O^T[m, s] = Wo_chunk.T @ A^T chunk accumulated over 4 chunks -> fp32.
Host: out[b] = (O^T_hg0 + O^T_hg1).T + bo.
"""

import sys

sys.path.insert(0, "/opt/trn_rl_repo")

import ml_dtypes
import numpy as np

import concourse.bacc as bacc
import concourse.mybir as mybir
from concourse import tile
from concourse.bass_utils import run_bass_kernel_spmd

F32 = mybir.dt.float32
BF16 = mybir.dt.bfloat16
AF = mybir.ActivationFunctionType
NP_BF16 = ml_dtypes.bfloat16

H, DK, DV, DM = 16, 64, 64, 1024
B, S = 4, 2048
HL = H // 2          # heads per core
NB = HL * DK         # 512: per-core projection width
NDM = DM // 128      # 8 contraction chunks
NT = NB // 128       # 4 row-tiles of QT/KT/AT
NKT = S // 128       # 16 k-tiles
NQB = S // 1024      # 2 q-blocks of 1024
SCALE = 1.0 / 8.0    # 1/sqrt(DK)

_CACHED_NC = None

import os

DEBUG_DUMP = bool(os.environ.get("KERNEL_DEBUG_DUMP"))


def _build():
    nc = bacc.Bacc("TRN2", debug=False)

    qT = nc.dram_tensor("qT", [DM, S], BF16, kind="ExternalInput")
    kT = nc.dram_tensor("kT", [DM, S], BF16, kind="ExternalInput")
    vT = nc.dram_tensor("vT", [DM, S], BF16, kind="ExternalInput")
    wq = nc.dram_tensor("wq", [DM, NB], BF16, kind="ExternalInput")
    wk = nc.dram_tensor("wk", [DM, NB], BF16, kind="ExternalInput")
    wv = nc.dram_tensor("wv", [DM, NB], BF16, kind="ExternalInput")
    wo = nc.dram_tensor("wo", [NB, DM], BF16, kind="ExternalInput")
    bq = nc.dram_tensor("bq", [NB], F32, kind="ExternalInput")
    bk = nc.dram_tensor("bk", [NB], F32, kind="ExternalInput")
    bv = nc.dram_tensor("bv", [NB], BF16, kind="ExternalInput")
    ones = nc.dram_tensor("ones", [1, 128], BF16, kind="ExternalInput")
    outT = nc.dram_tensor("outT", [DM, S], F32, kind="ExternalOutput")
    if DEBUG_DUMP:
        qt_dbg = nc.dram_tensor("qt_dbg", [128, NT, S], BF16, kind="ExternalOutput")
        kt_dbg = nc.dram_tensor("kt_dbg", [128, NT, S], BF16, kind="ExternalOutput")
        vp_dbg = nc.dram_tensor(
            "vp_dbg", [128, NKT, HL, DV + 1], BF16, kind="ExternalOutput"
        )
        at_dbg = nc.dram_tensor("at_dbg", [128, NT, S], BF16, kind="ExternalOutput")
        ex_dbg = nc.dram_tensor("ex_dbg", [128, 1024], BF16, kind="ExternalOutput")
        pv_dbg = nc.dram_tensor("pv_dbg", [128, 1024], F32, kind="ExternalOutput")

    with tile.TileContext(nc) as tc:
        with tc.tile_pool(name="persist", bufs=1) as persist:
            QT = persist.tile([128, NT, S], BF16)
            KT = persist.tile([128, NT, S], BF16)
            Vp = persist.tile([128, NKT, HL, DV + 1], BF16)
            wo_sb = persist.tile([128, NT, DM], BF16)
            bq_sb = persist.tile([128, NT], F32)
            bk_sb = persist.tile([128, NT], F32)
            bv_sb = persist.tile([1, NB], BF16)
            ones_sb = persist.tile([1, 128], BF16)

            nc.sync.dma_start(wo_sb[:], wo.rearrange("(c p) m -> p c m", p=128))
            nc.sync.dma_start(bq_sb[:], bq.rearrange("(t p) -> p t", p=128))
            nc.sync.dma_start(bk_sb[:], bk.rearrange("(t p) -> p t", p=128))
            nc.sync.dma_start(bv_sb[:], bv.rearrange("(o n) -> o n", o=1))
            nc.sync.dma_start(ones_sb[:], ones[:])
            nc.vector.memset(Vp[:, :, :, DV : DV + 1], 1.0)

            # ---- Stage 1: projections ----
            with (
                tc.tile_pool(name="acts", bufs=3) as acts_pool,
                tc.tile_pool(name="wgt", bufs=2) as wgt_pool,
                tc.tile_pool(name="ps_proj", bufs=2, space="PSUM") as ps_proj,
            ):
                for src_act, src_w, bias_sb, dstT in (
                    (qT, wq, bq_sb, QT),
                    (kT, wk, bk_sb, KT),
                ):
                    wt = wgt_pool.tile([128, NDM, NB], BF16, tag="w")
                    nc.sync.dma_start(
                        wt[:], src_w.rearrange("(c p) n -> p c n", p=128)
                    )
                    for sq in range(NQB):
                        act = acts_pool.tile([128, NDM, 1024], BF16, tag="a")
                        nc.sync.dma_start(
                            act[:],
                            src_act[:, sq * 1024 : (sq + 1) * 1024].rearrange(
                                "(c p) s -> p c s", p=128
                            ),
                        )
                        for t in range(NT):
                            ps = ps_proj.tile([128, 1024], F32, tag="ps")
                            for half in range(2):
                                hs = slice(half * 512, half * 512 + 512)
                                for c in range(NDM):
                                    nc.tensor.matmul(
                                        ps[:, hs],
                                        wt[:, c, t * 128 : (t + 1) * 128],
                                        act[:, c, hs],
                                        start=(c == 0),
                                        stop=(c == NDM - 1),
                                    )
                            nc.vector.tensor_scalar_add(
                                dstT[:, t, sq * 1024 : (sq + 1) * 1024],
                                ps[:],
                                bias_sb[:, t : t + 1],
                            )

                # V projection: natural orientation, bias via ones-row matmul
                wt = wgt_pool.tile([128, NDM, NB], BF16, tag="w")
                nc.sync.dma_start(wt[:], wv.rearrange("(c p) n -> p c n", p=128))
                for sq in range(NQB):
                    act = acts_pool.tile([128, NDM, 1024], BF16, tag="a")
                    nc.sync.dma_start(
                        act[:],
                        vT[:, sq * 1024 : (sq + 1) * 1024].rearrange(
                            "(c p) s -> p c s", p=128
                        ),
                    )
                    for sti in range(8):
                        st = sq * 8 + sti
                        ps = ps_proj.tile([128, 512], F32, tag="ps")
                        for c in range(NDM):
                            nc.tensor.matmul(
                                ps[:],
                                act[:, c, sti * 128 : (sti + 1) * 128],
                                wt[:, c, :],
                                start=(c == 0),
                                stop=False,
                            )
                        nc.tensor.matmul(
                            ps[:],
                            ones_sb[0:1, :],
                            bv_sb[0:1, :],
                            start=False,
                            stop=True,
                        )
                        nc.vector.tensor_copy(
                            Vp[:, st, :, 0:DV],
                            ps[:].rearrange("p (h d) -> p h d", h=HL),
                        )

            if DEBUG_DUMP:
                nc.sync.dma_start(qt_dbg[:], QT[:])
                nc.sync.dma_start(kt_dbg[:], KT[:])
                nc.sync.dma_start(vp_dbg[:], Vp[:])

            # ---- Stage 2: attention ----
            with tc.tile_pool(name="att", bufs=1) as att_pool:
                AT = att_pool.tile([128, NT, S], BF16)
                with (
                    tc.tile_pool(name="expS", bufs=34) as exp_pool,
                    tc.tile_pool(name="rec", bufs=3) as rec_pool,
                    tc.tile_pool(name="ps_sc", bufs=2, space="PSUM") as ps_sc,
                    tc.tile_pool(name="ps_pv", bufs=2, space="PSUM") as ps_pv,
                ):
                    for hp in range(HL // 2):  # head pairs: even head on
                        t = hp                 # partitions 0-63, odd on 64-127
                        for qb in range(NQB):
                            qsl = slice(qb * 1024, (qb + 1) * 1024)
                            # scores + exp for both heads of the pair first;
                            # the PV matmuls follow so the PE never waits on
                            # the exp of the tile it just produced.
                            ex_tiles = []
                            for kt in range(NKT):
                                for sub in range(2):
                                    psl = slice(sub * 64, sub * 64 + 64)
                                    scp = ps_sc.tile([128, 1024], F32, tag="sc")
                                    for half in range(2):
                                        hs = slice(
                                            qb * 1024 + half * 512,
                                            qb * 1024 + half * 512 + 512,
                                        )
                                        nc.tensor.matmul(
                                            scp[:, half * 512 : half * 512 + 512],
                                            KT[psl, t, kt * 128 : (kt + 1) * 128],
                                            QT[psl, t, hs],
                                            start=True,
                                            stop=True,
                                        )
                                    ex = exp_pool.tile([128, 1024], BF16, tag="e")
                                    nc.scalar.activation(
                                        ex[:], scp[:], AF.Exp, scale=SCALE
                                    )
                                    if DEBUG_DUMP and hp == 0 and qb == 0 and kt == 0 and sub == 0:
                                        nc.sync.dma_start(ex_dbg[:], ex[:])
                                    ex_tiles.append(ex)
                            pv_list = [
                                ps_pv.tile([128, 1024], F32, tag="pv", name=f"pv{i}")
                                for i in range(2)
                            ]
                            for kt in range(NKT):
                                for sub in range(2):
                                    for half in range(2):
                                        hs = slice(half * 512, half * 512 + 512)
                                        nc.tensor.matmul(
                                            pv_list[sub][0 : DV + 1, hs],
                                            Vp[:, kt, hp * 2 + sub, :],
                                            ex_tiles[kt * 2 + sub][:, hs],
                                            start=(kt == 0),
                                            stop=(kt == NKT - 1),
                                        )
                            if DEBUG_DUMP and hp == 0 and qb == 0:
                                pv_stg = rec_pool.tile(
                                    [128, 1024], F32, tag="pvstg"
                                )
                                nc.vector.tensor_copy(pv_stg[:], pv_list[0][:])
                                nc.sync.dma_start(pv_dbg[:], pv_stg[:])
                            for sub in range(2):
                                psl = slice(sub * 64, sub * 64 + 64)
                                pvp = pv_list[sub]
                                rec = rec_pool.tile([1, 1024], F32, tag="r")
                                recb = rec_pool.tile([64, 1024], F32, tag="rb")
                                dcp = rec_pool.tile([1, 1024], F32, tag="d")
                                # custom-DVE ucode mishandles base_partition=64
                                # PSUM reads in-kernel; stage through partition 0
                                nc.vector.tensor_copy(dcp[:], pvp[DV : DV + 1, :])
                                nc.vector.reciprocal_approx_fast(rec[:], dcp[:])
                                nc.gpsimd.partition_broadcast(recb[:], rec[:])
                                nc.vector.tensor_mul(
                                    AT[psl, t, qsl], pvp[0:DV, :], recb[:]
                                )

                if DEBUG_DUMP:
                    nc.sync.dma_start(at_dbg[:], AT[:])

                # ---- Stage 3: output projection ----
                with (
                    tc.tile_pool(name="ostage", bufs=3) as ostage,
                    tc.tile_pool(name="ps_o", bufs=2, space="PSUM") as ps_o,
                ):
                    for m in range(NDM):
                        for sbk in range(NQB):
                            ps = ps_o.tile([128, 1024], F32, tag="po")
                            for half in range(2):
                                hs = slice(
                                    sbk * 1024 + half * 512,
                                    sbk * 1024 + half * 512 + 512,
                                )
                                for cc in range(NT):
                                    nc.tensor.matmul(
                                        ps[:, half * 512 : half * 512 + 512],
                                        wo_sb[:, cc, m * 128 : (m + 1) * 128],
                                        AT[:, cc, hs],
                                        start=(cc == 0),
                                        stop=(cc == NT - 1),
                                    )
                            ot = ostage.tile([128, 1024], F32, tag="o")
                            nc.vector.tensor_copy(ot[:], ps[:])
                            nc.sync.dma_start(
                                outT[
                                    m * 128 : (m + 1) * 128,
                                    sbk * 1024 : (sbk + 1) * 1024,
                                ],
                                ot[:],
                            )

    nc.compile()
    return nc


def get_nc():
    global _CACHED_NC
    if _CACHED_NC is None:
        _CACHED_NC = _build()
    return _CACHED_NC


def _bf(x):
    return np.ascontiguousarray(np.asarray(x, np.float32)).astype(NP_BF16)


def make_in_maps(queries, keys, values, Wq, bq, Wk, bk, Wv, bv, Wo, bo):
    queries = np.asarray(queries, np.float32)
    keys = np.asarray(keys, np.float32)
    values = np.asarray(values, np.float32)
    Wq = np.asarray(Wq, np.float32)
    Wk = np.asarray(Wk, np.float32)
    Wv = np.asarray(Wv, np.float32)
    Wo = np.asarray(Wo, np.float32)
    bq = np.asarray(bq, np.float32)
    bk = np.asarray(bk, np.float32)
    bv = np.asarray(bv, np.float32)
    ones = np.ones((1, 128), NP_BF16)
    in_maps = []
    for core in range(8):
        b, hg = divmod(core, 2)
        sl = slice(hg * NB, (hg + 1) * NB)
        in_maps.append(
            {
                "qT": _bf(queries[b].T),
                "kT": _bf(keys[b].T),
                "vT": _bf(values[b].T),
                "wq": _bf(Wq[:, sl]),
                "wk": _bf(Wk[:, sl]),
                "wv": _bf(Wv[:, sl]),
                "wo": _bf(Wo[sl, :]),
                "bq": np.ascontiguousarray(bq[sl]),
                "bk": np.ascontiguousarray(bk[sl]),
                "bv": _bf(bv[sl]),
                "ones": ones,
            }
        )
    return in_maps


def assemble(results, bo):
    bo = np.asarray(bo, np.float32)
    out = np.empty((B, S, DM), np.float32)
    for b in range(B):
        acc = results[2 * b]["outT"] + results[2 * b + 1]["outT"]
        out[b] = acc.T + bo
    return out


def run(trace=False, **inputs):
    if trace:
        # NTFF profiling shim: this image's antenv lacks axon_hooks.
        import types

        try:
            from antenv import axon_hooks  # noqa: F401
        except ImportError:
            from trn_agent_boot.trn_boot import _ntff_profile_via_ctypes

            mod = types.ModuleType("antenv.axon_hooks")
            _hook = _ntff_profile_via_ctypes("/opt/axon/libaxon_pjrt.so")
            mod.get_axon_ntff_profile_hook = lambda: _hook
            sys.modules["antenv.axon_hooks"] = mod
    nc = get_nc()
    bo = inputs["bo"]
    in_maps = make_in_maps(**inputs)
    res = run_bass_kernel_spmd(nc, in_maps, list(range(8)), trace=trace)
    return assemble(res.results, bo), res


def kernel(**inputs):
    out, _ = run(trace=False, **inputs)
    return out
